# revision 1
# baseline (speedup 1.0000x reference)
"""Distributed Trainium2 kernel for nn_Attention (RMSNorm + QKV + RoPE +
causal SDPA + out-proj) over 8 NeuronCores.

v4 strategy (project-locally-then-AllToAll): each core c
  phase 0: RMSNorm of its own 512-token chunk in transposed layout
           (features on partitions); norm_w folded into the weight casts,
           1/rms folded into the local RoPE tables (RoPE commutes with
           per-token scalars) and one multiply for V.
  phase 1: Q/K/V projections of the LOCAL chunk for ALL 16 heads
           (same FLOPs as two-head x all-tokens, but needs no collective
           first - starts at ~10us), RoPE applied locally.
  A2A #1:  AllToAll redistributes [pair-block q|k|v] from token-sharded
           to head-sharded (3MB/rank) - also the launch-skew rendezvous.
  phase 4: causal SDPA in S^T layout for the core's two heads:
           scores^T = K_blk^T.T @ Q^T (heads row-packed on the PE array),
           exp on ScalarE (no max subtraction - scores are O(6)),
           diag-block masking, AV matmul with a ones column appended to V
           so the softmax denominator accumulates in the same PSUM tile;
           deferred division. The two batches are interleaved so one
           batch's exp overlaps the other's matmuls.
  A2A #2:  context back to token-sharded (1MB/rank).
  phase 6: out-projection for the core's own 512-token chunk.
Host does layout-only prep (transpose, head-column permutation, constant
RoPE/mask tables) and the final concat.
"""
import sys

sys.path.insert(0, "/opt/trn_rl_repo")

import numpy as np
import ml_dtypes
from contextlib import ExitStack

import concourse.bass as bass
import concourse.mybir as mybir
import concourse.tile as tile
from concourse import bacc
from concourse.bass_utils import run_bass_kernel_spmd
from concourse.masks import make_identity

F32 = mybir.dt.float32
BF16 = mybir.dt.bfloat16

B, S, D, H, DH = 2, 2048, 1024, 16, 64
NC = 8
TOK = B * S            # 4096
CHUNK = TOK // NC      # 512
EPS = 1.1920929e-07
THETA = 10000.0
NKB = S // 128         # key blocks per batch: 16
QT = S // 512          # q tiles per batch: 4

_CACHE = {}
DEBUG = False


def _build():
    nc = bacc.Bacc("TRN2", target_bir_lowering=False, debug=False, num_devices=NC)

    xt_d = nc.dram_tensor("xt", [D, CHUNK], F32, kind="ExternalInput")
    nw_d = nc.dram_tensor("nw", [D, 1], F32, kind="ExternalInput")
    wq_d = nc.dram_tensor("wqc", [D, D], F32, kind="ExternalInput")
    wk_d = nc.dram_tensor("wkc", [D, D], F32, kind="ExternalInput")
    wv_d = nc.dram_tensor("wvc", [D, D], F32, kind="ExternalInput")
    wo_d = nc.dram_tensor("wo", [D, D], F32, kind="ExternalInput")
    cos_d = nc.dram_tensor("cosb", [128, CHUNK], BF16, kind="ExternalInput")
    sin_d = nc.dram_tensor("sinb", [128, CHUNK], BF16, kind="ExternalInput")
    msk_d = nc.dram_tensor("dmask", [128, 128], BF16, kind="ExternalInput")
    out_d = nc.dram_tensor("out", [CHUNK, D], F32, kind="ExternalOutput")

    # A2A #1: block p = [q-pair-p(128) | k-pair-p(128) | v-pair-p(128)] of
    # OUR chunk; received block i = our pair's q/k/v for chunk i.
    aq_in = nc.dram_tensor("aq_in", [NC * 384, CHUNK], BF16)
    aq_out = nc.dram_tensor("aq_out", [NC * 384, CHUNK], BF16)
    a2a_in = nc.dram_tensor("a2a_in", [NC * 128, CHUNK], BF16)
    a2a_out = nc.dram_tensor("a2a_out", [NC * 128, CHUNK], BF16)

    with tile.TileContext(nc) as tc, ExitStack() as ctx:
        pp = ctx.enter_context(tc.tile_pool(name="persist", bufs=1))

        # ---- persistent tiles ----
        qT = pp.tile([128, TOK], BF16, tag="qT")
        kT = pp.tile([128, TOK], BF16, tag="kT")
        vT = pp.tile([128, TOK], BF16, tag="vT")
        v_all = pp.tile([128, B * NKB * 130], BF16, tag="v_all")
        cosT = pp.tile([128, CHUNK], BF16, tag="cosT")
        sinT = pp.tile([128, CHUNK], BF16, tag="sinT")
        dmaskT = pp.tile([128, 128], BF16, tag="dmaskT")
        identT = pp.tile([128, 128], BF16, tag="identT")
        ones128 = pp.tile([128, 1], BF16, tag="ones128")
        ones1 = pp.tile([1, 128], BF16, tag="ones1")
        nw_sb = pp.tile([128, 8], F32, tag="nw_sb")
        wo_sb = pp.tile([128, 8, 1024], BF16, tag="wo_sb")
        ctx_sb = pp.tile([128, TOK], BF16, tag="ctx_sb")

        nc.vector.memset(ones128, 1.0)
        nc.vector.memset(ones1, 1.0)
        for kt in range(8):
            nc.sync.dma_start(out=nw_sb[:, kt : kt + 1], in_=nw_d[kt * 128 : (kt + 1) * 128, :])

        # ---- phase 0: local RMSNorm of own chunk (transposed layout) ----
        xn_cm = tc.tile_pool(name="xnc", bufs=1)
        xn_pool = xn_cm.__enter__()
        xn_sb = []
        with tc.tile_pool(name="rms", bufs=2) as rms_pool, \
             tc.tile_pool(name="ps0", bufs=1, space="PSUM") as ps0:
            ssq = ps0.tile([1, CHUNK], F32, tag="ssq")
            xbs = []
            for kt in range(8):
                xtile = rms_pool.tile([128, CHUNK], F32, tag=f"xt{kt}")
                nc.sync.dma_start(out=xtile[0:64, :], in_=xt_d[kt * 128 : kt * 128 + 64, :])
                nc.sync.dma_start(out=xtile[64:128, :], in_=xt_d[kt * 128 + 64 : (kt + 1) * 128, :])
                xbs.append(xtile)
                xsq = rms_pool.tile([128, CHUNK], BF16, tag="xsq")
                nc.vector.tensor_mul(xsq, xtile, xtile)
                nc.tensor.matmul(ssq, ones128, xsq, start=(kt == 0), stop=(kt == 7))
            eps_t = rms_pool.tile([1, 1], F32, tag="eps_t")
            nc.vector.memset(eps_t, float(EPS))
            rstd = rms_pool.tile([1, CHUNK], F32, tag="rstd")
            nc.scalar.activation(rstd, ssq, mybir.ActivationFunctionType.Sqrt,
                                 bias=eps_t[0:1, 0:1], scale=1.0 / D)
            inv = rms_pool.tile([1, CHUNK], F32, tag="inv")
            nc.vector.reciprocal_approx_fast(out=inv, in_=rstd)
            invb = rms_pool.tile([1, CHUNK], BF16, tag="invb")
            nc.vector.tensor_copy(invb, inv)
            rb = ps0.tile([128, CHUNK], F32, tag="rb")
            nc.tensor.matmul(rb, ones1, invb, start=True, stop=True)
            # xn = (x * nw) * 1/rms  (normalized local chunk, bf16)
            for kt in range(8):
                xn = xn_pool.tile([128, CHUNK], BF16, name=f"xn{kt}", tag=f"xn{kt}")
                nc.vector.scalar_tensor_tensor(
                    out=xn, in0=xbs[kt], scalar=nw_sb[:, kt : kt + 1], in1=rb,
                    op0=mybir.AluOpType.mult, op1=mybir.AluOpType.mult)
                xn_sb.append(xn)
            nc.sync.dma_start(out=cosT, in_=cos_d[:, :])
            nc.sync.dma_start(out=sinT, in_=sin_d[:, :])

        nc.sync.dma_start(out=dmaskT, in_=msk_d[:, :])
        make_identity(nc, identT)

        # ---- weight staging + cast (norm_w folded); q/k on ACT, v on DVE ----
        w_cm = tc.tile_pool(name="wpool", bufs=1)
        w_pool = w_cm.__enter__()
        w_sb = {p: w_pool.tile([128, 8, 1024], BF16, name=f"w_{p}", tag=f"w_{p}")
                for p in "qkv"}
        with tc.tile_pool(name="wstage", bufs=4) as wstage:
            for p, d in (("q", wq_d), ("k", wk_d), ("v", wv_d)):
                for kt in range(8):
                    st = wstage.tile([128, 1024], F32, tag="wst")
                    nc.sync.dma_start(out=st, in_=d[kt * 128 : (kt + 1) * 128, :])
                    for h in range(2):
                        sl = slice(h * 512, (h + 1) * 512)
                        if p == "v":
                            nc.vector.tensor_scalar_mul(
                                w_sb[p][:, kt, sl], st[:, sl], nw_sb[:, kt : kt + 1])
                        else:
                            nc.scalar.activation(
                                w_sb[p][:, kt, sl], st[:, sl],
                                mybir.ActivationFunctionType.Copy,
                                scale=nw_sb[:, kt : kt + 1])

            for kt in range(8):
                st2 = wstage.tile([128, 1024], F32, tag="wost")
                nc.sync.dma_start(out=st2[0:64, :], in_=wo_d[kt * 128 : kt * 128 + 64, :])
                nc.sync.dma_start(out=st2[64:128, :], in_=wo_d[kt * 128 + 64 : (kt + 1) * 128, :])
                nc.vector.tensor_copy(wo_sb[:, kt, :], st2)

            # ---- phase 1: V projections for ALL pairs first (feeds the
            # early A2A that doubles as the launch-skew rendezvous), then
            # Q/K projections + RoPE under that collective ----
            with tc.tile_pool(name="pstage", bufs=6) as pstage, \
                 tc.tile_pool(name="psproj", bufs=2, space="PSUM") as psproj:
                for p in range(NC):
                    acc = {w: psproj.tile([128, CHUNK], F32, name=f"a{w}", tag=f"a{w}")
                           for w in "qk"}
                    for kt in range(8):
                        for w in "qk":
                            nc.tensor.matmul(
                                acc[w], w_sb[w][:, kt, p * 128 : (p + 1) * 128],
                                xn_sb[kt], start=(kt == 0), stop=(kt == 7))
                    for w, off in (("q", 0), ("k", 128)):
                        t = pstage.tile([128, CHUNK], BF16, tag=f"t{w}")
                        nc.scalar.copy(t, acc[w])
                        sw = pstage.tile([128, CHUNK], BF16, tag=f"sw{w}")
                        for a, b2 in ((0, 32), (64, 96)):
                            nc.sync.dma_start(out=sw[a : a + 32, :], in_=t[b2 : b2 + 32, :])
                            nc.sync.dma_start(out=sw[b2 : b2 + 32, :], in_=t[a : a + 32, :])
                        nc.vector.tensor_mul(t, t, cosT)
                        nc.vector.tensor_mul(sw, sw, sinT)
                        nc.vector.tensor_add(t, t, sw)
                        base_r = p * 384 + off
                        nc.sync.dma_start(out=aq_in[base_r : base_r + 64, :], in_=t[0:64, :])
                        nc.sync.dma_start(out=aq_in[base_r + 64 : base_r + 128, :], in_=t[64:128, :])
                for p in range(NC):
                    accv = psproj.tile([128, CHUNK], F32, name="av", tag="av")
                    for kt in range(8):
                        nc.tensor.matmul(
                            accv, w_sb["v"][:, kt, p * 128 : (p + 1) * 128],
                            xn_sb[kt], start=(kt == 0), stop=(kt == 7))
                    vp = pstage.tile([128, CHUNK], BF16, tag="vp")
                    nc.vector.tensor_copy(vp, accv)
                    vb = p * 384 + 256
                    nc.sync.dma_start(out=aq_in[vb : vb + 64, :], in_=vp[0:64, :])
                    nc.sync.dma_start(out=aq_in[vb + 64 : vb + 128, :], in_=vp[64:128, :])

        # ---- A2A #1: q/k/v token-sharded -> head-sharded ----
        nc.gpsimd.collective_compute(
            "AllToAll", mybir.AluOpType.bypass,
            replica_groups=[list(range(NC))],
            ins=[aq_in.ap().opt()], outs=[aq_out.ap().opt()])
        w_cm.__exit__(None, None, None)
        xn_cm.__exit__(None, None, None)

        # unpack per chunk (V first) with that chunk's V-transposes emitted
        # immediately behind it, so transposes stream during the unpack window
        with tc.tile_pool(name="psvtr", bufs=3, space="PSUM") as psvtr:
            vv = v_all.rearrange("p (blk c) -> p blk c", c=130)
            for i in range(NC):
                sl = slice(i * CHUNK, (i + 1) * CHUNK)
                for dst, off in ((vT, 256), (qT, 0), (kT, 128)):
                    nc.sync.dma_start(out=dst[0:64, sl],
                                      in_=aq_out[i * 384 + off : i * 384 + off + 64, :])
                    nc.sync.dma_start(out=dst[64:128, sl],
                                      in_=aq_out[i * 384 + off + 64 : i * 384 + off + 128, :])
                b = i // 4
                for kb in range(4 * (i % 4), 4 * (i % 4) + 4):
                    blk = b * NKB + kb
                    pst = psvtr.tile([128, 128], BF16, tag="vtr")
                    nc.tensor.transpose(pst, vT[:, b * S + kb * 128 : b * S + (kb + 1) * 128], identT)
                    nc.scalar.copy(vv[:, blk, 0:64], pst[:, 0:64])
                    nc.scalar.copy(vv[:, blk, 65:129], pst[:, 64:128])
            nc.gpsimd.memset(vv[:, :, 64:65], 1.0)
            nc.gpsimd.memset(vv[:, :, 129:130], 1.0)

        # ---- phase 4: SDPA (batches interleaved for PE/ACT overlap) ----
        with tc.tile_pool(name="pexp", bufs=6) as pexp, \
             tc.tile_pool(name="cnorm", bufs=2) as cnorm, \
             tc.tile_pool(name="ps4", bufs=3, space="PSUM") as ps4, \
             tc.tile_pool(name="ps4b", bufs=1, space="PSUM") as ps4b, \
             tc.tile_pool(name="ps4c", bufs=1, space="PSUM") as ps4c:
            for step in range(B * QT):
                b, j = step % B, step // B
                base = b * S
                ctxp = {0: ps4c.tile([65, 512], F32, name=f"ctxA{b}", tag=f"ctxA{b}"),
                        1: ps4c.tile([65, 512], F32, name=f"ctxB{b}", tag=f"ctxB{b}")}
                nkb = 4 * (j + 1)
                for kb in range(nkb):
                    m = kb - 4 * j
                    c0 = 128 * m if m >= 0 else 0
                    w = 512 - c0
                    qcol0 = base + 512 * j + c0
                    koff = base + kb * 128
                    for hi, r0 in ((0, 0), (1, 64)):
                        sc = ps4.tile([128, 512], F32, name="sc", tag="sc")
                        nc.tensor.matmul(
                            sc[:, 0:w],
                            kT[r0 : r0 + 64, koff : koff + 128],
                            qT[r0 : r0 + 64, qcol0 : qcol0 + w],
                            start=True, stop=True)
                        p = pexp.tile([128, 512], BF16, name=f"p{hi}", tag=f"p{hi}")
                        nc.scalar.activation(
                            p[:, c0:512], sc[:, 0:w],
                            mybir.ActivationFunctionType.Exp, scale=0.125)
                        if m >= 0:
                            nc.vector.tensor_mul(p[:, c0 : c0 + 128], p[:, c0 : c0 + 128], dmaskT)
                        vcol = (b * NKB + kb) * 130 + hi * 65
                        nc.tensor.matmul(
                            ctxp[hi][:, c0:512],
                            v_all[:, vcol : vcol + 65],
                            p[:, c0:512],
                            start=(kb == 0), stop=(kb == nkb - 1),
                            skip_group_check=True)
                # normalize: ctx / denom (denom = row 64 of ctx psum)
                for hi, r0 in ((0, 0), (1, 64)):
                    den_s = cnorm.tile([1, 512], F32, tag="den_s")
                    nc.vector.tensor_copy(den_s, ctxp[hi][64:65, :])
                    rec = cnorm.tile([1, 512], F32, tag="rec")
                    nc.vector.reciprocal_approx_fast(out=rec, in_=den_s)
                    recb = cnorm.tile([1, 512], BF16, tag="recb")
                    nc.vector.tensor_copy(recb, rec)
                    bc = ps4b.tile([64, 512], F32, name="bc", tag="bc")
                    nc.tensor.matmul(bc, ones1[0:1, 0:64], recb, start=True, stop=True)
                    cth = cnorm.tile([64, 512], F32, tag="cth")
                    nc.vector.tensor_copy(cth, ctxp[hi][0:64, :])
                    nc.vector.tensor_mul(
                        ctx_sb[r0 : r0 + 64, base + 512 * j : base + 512 * (j + 1)],
                        cth, bc)
                cch = b * QT + j
                nc.sync.dma_start(
                    out=a2a_in[cch * 128 : cch * 128 + 64, :],
                    in_=ctx_sb[0:64, cch * CHUNK : (cch + 1) * CHUNK])
                nc.sync.dma_start(
                    out=a2a_in[cch * 128 + 64 : (cch + 1) * 128, :],
                    in_=ctx_sb[64:128, cch * CHUNK : (cch + 1) * CHUNK])

        # ---- A2A #2: head-sharded ctx -> token-sharded ctx ----
        nc.gpsimd.collective_compute(
            "AllToAll", mybir.AluOpType.bypass,
            replica_groups=[list(range(NC))],
            ins=[a2a_in.ap().opt()], outs=[a2a_out.ap().opt()])

        # ---- phase 6: out-projection on own token chunk ----
        with tc.tile_pool(name="ctxgp", bufs=1) as ctxgp, \
             tc.tile_pool(name="outp", bufs=3) as outp, \
             tc.tile_pool(name="ps6", bufs=2, space="PSUM") as ps6:
            ctxg = ctxgp.tile([128, NC * CHUNK], BF16, tag="ctxg")
            for cb in range(NC):
                nc.sync.dma_start(out=ctxg[0:64, cb * CHUNK : (cb + 1) * CHUNK],
                                  in_=a2a_out[cb * 128 : cb * 128 + 64, :])
                nc.sync.dma_start(out=ctxg[64:128, cb * CHUNK : (cb + 1) * CHUNK],
                                  in_=a2a_out[cb * 128 + 64 : (cb + 1) * 128, :])
            for tl in range(4):
                pso = {nh: ps6.tile([128, 512], F32, name=f"op{nh}", tag=f"op{nh}") for nh in range(2)}
                for nh in range(2):
                    for cb in range(8):
                        nc.tensor.matmul(
                            pso[nh],
                            ctxg[:, cb * CHUNK + tl * 128 : cb * CHUNK + (tl + 1) * 128],
                            wo_sb[:, cb, nh * 512 : (nh + 1) * 512],
                            start=(cb == 0), stop=(cb == 7))
                ost = outp.tile([128, 1024], F32, tag="ost")
                nc.scalar.copy(ost[:, 0:512], pso[0])
                nc.scalar.copy(ost[:, 512:1024], pso[1])
                nc.sync.dma_start(out=out_d[tl * 128 : tl * 128 + 64, :], in_=ost[0:64, :])
                nc.sync.dma_start(out=out_d[tl * 128 + 64 : (tl + 1) * 128, :], in_=ost[64:128, :])

    nc.compile()
    return nc


def _head_cols(h, deinterleave):
    base = h * DH
    if deinterleave:
        return np.concatenate([base + np.arange(0, DH, 2), base + np.arange(1, DH, 2)])
    return base + np.arange(DH)


def _make_tables():
    inv_freq = 1.0 / (THETA ** (np.arange(0, DH, 2) / DH))   # [32]
    ang = np.arange(S)[:, None] * inv_freq[None, :]          # [2048, 32]
    ch = np.cos(ang).T.astype(np.float32)                    # [32, 2048]
    sh = np.sin(ang).T.astype(np.float32)
    cosb = np.tile(np.concatenate([ch, ch, ch, ch], axis=0), (1, B))
    sinb = np.tile(np.concatenate([-sh, sh, -sh, sh], axis=0), (1, B))
    kk, qq = np.meshgrid(np.arange(128), np.arange(128), indexing="ij")
    dmask = (kk <= qq).astype(np.float32)
    bf = ml_dtypes.bfloat16
    return cosb.astype(bf), sinb.astype(bf), dmask.astype(bf)


def _in_maps(inputs):
    x = np.ascontiguousarray(inputs["x"], dtype=np.float32)
    norm_w = np.asarray(inputs["norm_w"], dtype=np.float32)
    wq = np.asarray(inputs["wq"], dtype=np.float32)
    wk = np.asarray(inputs["wk"], dtype=np.float32)
    wv = np.asarray(inputs["wv"], dtype=np.float32)
    wo = np.ascontiguousarray(inputs["wo"], dtype=np.float32)

    xT = np.ascontiguousarray(x.reshape(TOK, D).T)           # [1024, 4096]
    cosb, sinb, dmask = _make_tables()
    nw = np.ascontiguousarray(norm_w.reshape(D, 1))

    qcols = np.concatenate([_head_cols(h, True) for h in range(H)])
    vcols = np.concatenate([_head_cols(h, False) for h in range(H)])
    wqf = np.ascontiguousarray(wq[:, qcols])
    wkf = np.ascontiguousarray(wk[:, qcols])
    wvf = np.ascontiguousarray(wv[:, vcols])

    maps = []
    for c in range(NC):
        maps.append({
            "xt": np.ascontiguousarray(xT[:, c * CHUNK : (c + 1) * CHUNK]),
            "nw": nw,
            "wqc": wqf,
            "wkc": wkf,
            "wvc": wvf,
            "wo": wo,
            "cosb": np.ascontiguousarray(cosb[:, c * CHUNK : (c + 1) * CHUNK]),
            "sinb": np.ascontiguousarray(sinb[:, c * CHUNK : (c + 1) * CHUNK]),
            "dmask": dmask,
        })
    return maps


def _run(inputs, trace=False):
    if "nc" not in _CACHE:
        _CACHE["nc"] = _build()
    nc = _CACHE["nc"]
    res = run_bass_kernel_spmd(nc, _in_maps(inputs), core_ids=list(range(NC)),
                               trace=trace)
    chunks = [res.results[c]["out"] for c in range(NC)]
    out = np.concatenate(chunks, axis=0).reshape(B, S, D).astype(np.float32)
    return out, res


def kernel(**inputs) -> np.ndarray:
    out, _ = _run(inputs, trace=False)
    return out



# revision 8
# speedup vs baseline: 1.1362x; 1.1362x over previous
"""Distributed Trainium2 kernel for nn_Attention (RMSNorm + QKV + RoPE +
causal SDPA + out-proj) over 8 NeuronCores.

v5 strategy (head-sharded QKV, no input collective): every core receives
the FULL x (transposed, bf16) and projects q/k/v for only ITS two heads
over all 4096 tokens -- identical FLOPs to v4's local-chunk/all-heads
split, but the 3MB AllToAll after the projections disappears.

  phase 0: RMSNorm statistics for the core's own 512-token chunk only
           (squares on DVE, column-sum via ones-matmul); 1/rms is
           AllGather'ed (2KB) -- this tiny collective doubles as the
           launch-skew rendezvous and completes under the x DMA stream.
  phase 1: q/k for the 2 heads, all tokens ([128,512] psum tiles,
           8 k-tiles each); RoPE applied at psum-evict time with
           cos/sin tables pre-scaled by 1/rms (RoPE commutes with
           per-token scalars).  v is projected directly TRANSPOSED
           (tokens on partitions: lhsT = x-block, rhs = wv) so SDPA
           needs no PE transposes; 1/rms applied per-partition at the
           v evict.  norm_w is folded into the weights on the host;
           all weights arrive pre-cast to bf16 (no on-chip casts).
  phase 2: causal SDPA in S^T layout for the two heads (v4 scheme:
           scores^T = K_blk.T @ Q, exp on ScalarE without max
           subtraction, diag-block masking, AV with a ones column in
           V so the softmax denominator rides in the same psum tile,
           deferred division).  Batches interleaved for PE/ACT overlap.
  A2A:     context head-sharded -> token-sharded (1MB/rank).
  phase 3: out-projection for the core's own 512-token chunk.
Host does layout-only prep (transpose+bf16 cast, head-column
permutation, norm_w fold, constant RoPE/mask tables) and final concat.
"""
import sys

sys.path.insert(0, "/opt/trn_rl_repo")

import numpy as np
import ml_dtypes
from contextlib import ExitStack

import concourse.bass as bass
import concourse.mybir as mybir
import concourse.tile as tile
from concourse import bacc
from concourse.bass_utils import run_bass_kernel_spmd

F32 = mybir.dt.float32
BF16 = mybir.dt.bfloat16

B, S, D, H, DH = 2, 2048, 1024, 16, 64
NC = 8
TOK = B * S            # 4096
CHUNK = TOK // NC      # 512
EPS = 1.1920929e-07
THETA = 10000.0
NKB = S // 128         # key blocks per batch: 16
QT = S // 512          # q tiles per batch: 4

_CACHE = {}
DEBUG = False


def _build():
    nc = bacc.Bacc("TRN2", target_bir_lowering=False, debug=False, num_devices=NC)

    xc_d = nc.dram_tensor("xc", [D, CHUNK], BF16, kind="ExternalInput")
    xtb_d = nc.dram_tensor("xtb", [D, TOK], BF16, kind="ExternalInput")
    wq_d = nc.dram_tensor("wqc", [D, 128], BF16, kind="ExternalInput")
    wk_d = nc.dram_tensor("wkc", [D, 128], BF16, kind="ExternalInput")
    wv_d = nc.dram_tensor("wvc", [D, 128], BF16, kind="ExternalInput")
    wo_d = nc.dram_tensor("wo", [D, D], BF16, kind="ExternalInput")
    cos_d = nc.dram_tensor("cosb", [128, TOK], BF16, kind="ExternalInput")
    sin_d = nc.dram_tensor("sinb", [128, TOK], BF16, kind="ExternalInput")
    msk_d = nc.dram_tensor("dmask", [128, 128], BF16, kind="ExternalInput")
    out_d = nc.dram_tensor("out", [CHUNK, D], F32, kind="ExternalOutput")

    if DEBUG:
        qTd = nc.dram_tensor("qTd", [128, TOK], BF16, kind="ExternalOutput")
        kTd = nc.dram_tensor("kTd", [128, TOK], BF16, kind="ExternalOutput")
        vvd = nc.dram_tensor("vvd", [128, 32 * 130], BF16, kind="ExternalOutput")
        ctxd = nc.dram_tensor("ctxd", [128, TOK], BF16, kind="ExternalOutput")
        invd = nc.dram_tensor("invd", [1, TOK], F32, kind="ExternalOutput")
    ag_in = nc.dram_tensor("ag_in", [1, CHUNK], F32)
    ag_out = nc.dram_tensor("ag_out", [1, TOK], F32)
    a2a_in = nc.dram_tensor("a2a_in", [NC * 128, CHUNK], BF16)
    a2a_out = nc.dram_tensor("a2a_out", [NC * 128, CHUNK], BF16)

    with tile.TileContext(nc) as tc, ExitStack() as ctx:
        pp = ctx.enter_context(tc.tile_pool(name="persist", bufs=1))

        # ---- persistent tiles ----
        qT = pp.tile([128, TOK], BF16, tag="qT")
        kT = pp.tile([128, TOK], BF16, tag="kT")
        # vv[:, blk, :] = [h0 dims 0:64 | ones | h1 dims 65:129 | ones]
        vv = pp.tile([128, B * NKB, 130], BF16, tag="vv")
        cosS = pp.tile([128, TOK], BF16, tag="cosS")
        sinS = pp.tile([128, TOK], BF16, tag="sinS")
        ctx_sb = pp.tile([128, TOK], BF16, tag="ctx_sb")
        wq_sb = pp.tile([128, 8, 128], BF16, tag="wq_sb")
        wk_sb = pp.tile([128, 8, 128], BF16, tag="wk_sb")
        wv_sb = pp.tile([128, 8, 128], BF16, tag="wv_sb")
        wo_sb = pp.tile([128, 8, 1024], BF16, tag="wo_sb")
        inv_all = pp.tile([1, TOK], F32, tag="inv_all")
        invT = pp.tile([128, B * NKB], F32, tag="invT")
        dmaskT = pp.tile([128, 128], BF16, tag="dmaskT")
        ones128 = pp.tile([128, 1], BF16, tag="ones128")
        ones1 = pp.tile([1, 128], BF16, tag="ones1")

        nc.vector.memset(ones128, 1.0)
        nc.vector.memset(ones1, 1.0)
        nc.gpsimd.memset(vv[:, :, 64:65], 1.0)
        nc.gpsimd.memset(vv[:, :, 129:130], 1.0)

        xs_cm = tc.tile_pool(name="xspool", bufs=1)
        xs_pool = xs_cm.__enter__()
        xs = xs_pool.tile([128, 8, TOK], BF16, tag="xs")
        xcs = xs_pool.tile([128, 8, CHUNK], BF16, tag="xcs")

        # ---- input DMAs (big 3D-AP transfers; issue in consumption order)
        nc.sync.dma_start(
            out=xcs, in_=xc_d.ap().rearrange("(k p) t -> p k t", p=128))
        nc.scalar.dma_start(
            out=wq_sb, in_=wq_d.ap().rearrange("(k p) d -> p k d", p=128))
        nc.scalar.dma_start(
            out=wk_sb, in_=wk_d.ap().rearrange("(k p) d -> p k d", p=128))
        nc.scalar.dma_start(
            out=wv_sb, in_=wv_d.ap().rearrange("(k p) d -> p k d", p=128))
        for tt in range(NC):
            sl = slice(tt * CHUNK, (tt + 1) * CHUNK)
            nc.sync.dma_start(
                out=xs[:, :, sl],
                in_=xtb_d.ap()[:, sl].rearrange("(k p) t -> p k t", p=128))
        nc.scalar.dma_start(out=cosS, in_=cos_d[:, :])
        nc.scalar.dma_start(out=sinS, in_=sin_d[:, :])
        nc.scalar.dma_start(out=dmaskT, in_=msk_d[:, :])
        nc.gpsimd.dma_start(
            out=wo_sb, in_=wo_d.ap().rearrange("(k p) d -> p k d", p=128))

        # ---- phase 0: RMSNorm stats of own chunk + AllGather of 1/rms ----
        with tc.tile_pool(name="rms", bufs=2) as rms_pool, \
             tc.tile_pool(name="psrms", bufs=1, space="PSUM") as psrms:
            ssq = psrms.tile([1, CHUNK], F32, tag="ssq")
            for kt in range(8):
                xsq = rms_pool.tile([128, CHUNK], BF16, tag="xsq")
                nc.vector.tensor_mul(xsq, xcs[:, kt, :], xcs[:, kt, :])
                nc.tensor.matmul(ssq, ones128, xsq, start=(kt == 0), stop=(kt == 7))
            eps_t = rms_pool.tile([1, 1], F32, tag="eps_t")
            nc.vector.memset(eps_t, float(EPS))
            rstd = rms_pool.tile([1, CHUNK], F32, tag="rstd")
            nc.scalar.activation(rstd, ssq, mybir.ActivationFunctionType.Sqrt,
                                 bias=eps_t[0:1, 0:1], scale=1.0 / D)
            inv = rms_pool.tile([1, CHUNK], F32, tag="inv")
            nc.vector.reciprocal_approx_fast(out=inv, in_=rstd)
            nc.sync.dma_start(out=ag_in[:, :], in_=inv)

        nc.gpsimd.collective_compute(
            "AllGather", mybir.AluOpType.bypass,
            replica_groups=[list(range(NC))],
            ins=[ag_in.ap().opt()], outs=[ag_out.ap().opt()])
        nc.sync.dma_start(out=inv_all, in_=ag_out[:, :])
        nc.sync.dma_start(
            out=invT, in_=ag_out.ap().rearrange("a (k p) -> p (a k)", p=128))

        # scale RoPE tables by 1/rms (per-token column scalar commutes with
        # RoPE): rb = broadcast of 1/rms over partitions, then in-place muls
        with tc.tile_pool(name="rbp", bufs=2) as rbpool:
            for tt in range(NC):
                sl = slice(tt * CHUNK, (tt + 1) * CHUNK)
                rb = rbpool.tile([128, CHUNK], F32, tag="rb")
                nc.gpsimd.partition_broadcast(rb[:, :], inv_all[0:1, sl])
                nc.vector.tensor_mul(cosS[:, sl], cosS[:, sl], rb)
                nc.vector.tensor_mul(sinS[:, sl], sinS[:, sl], rb)

            # ---- phase 1: QKV for the 2 heads over all tokens ----
            with tc.tile_pool(name="pstage", bufs=6) as pstage, \
                 tc.tile_pool(name="psqk", bufs=3, space="PSUM") as psqk, \
                 tc.tile_pool(name="psv", bufs=2, space="PSUM") as psv:
                for tt in range(NC):
                    sl = slice(tt * CHUNK, (tt + 1) * CHUNK)
                    for wsb, dst in ((wq_sb, qT), (wk_sb, kT)):
                        acc = psqk.tile([128, CHUNK], F32, tag="accqk")
                        for kt in range(8):
                            nc.tensor.matmul(acc, wsb[:, kt, :], xs[:, kt, sl],
                                             start=(kt == 0), stop=(kt == 7))
                        t = pstage.tile([128, CHUNK], BF16, tag="t")
                        nc.scalar.copy(t, acc)
                        sw = pstage.tile([128, CHUNK], BF16, tag="sw")
                        for a, b2 in ((0, 32), (64, 96)):
                            nc.gpsimd.dma_start(out=sw[a : a + 32, :], in_=t[b2 : b2 + 32, :])
                            nc.gpsimd.dma_start(out=sw[b2 : b2 + 32, :], in_=t[a : a + 32, :])
                        t1 = pstage.tile([128, CHUNK], BF16, tag="t1")
                        nc.vector.tensor_mul(t1, t, cosS[:, sl])
                        nc.vector.tensor_mul(sw, sw, sinS[:, sl])
                        nc.vector.tensor_add(dst[:, sl], t1, sw)
                    # v transposed directly: lhsT = x token-block (stationary)
                    for vb in range(4):
                        blk = tt * 4 + vb
                        c0 = tt * CHUNK + vb * 128
                        accv = psv.tile([128, 128], F32, tag="accv")
                        for kt in range(8):
                            nc.tensor.matmul(accv, xs[:, kt, c0 : c0 + 128],
                                             wv_sb[:, kt, :],
                                             start=(kt == 0), stop=(kt == 7))
                        nc.vector.tensor_scalar_mul(
                            vv[:, blk, 0:64], accv[:, 0:64], invT[:, blk : blk + 1])
                        nc.vector.tensor_scalar_mul(
                            vv[:, blk, 65:129], accv[:, 64:128], invT[:, blk : blk + 1])

        xs_cm.__exit__(None, None, None)
        vvf = vv.rearrange("p blk c -> p (blk c)")

        # ---- phase 2: SDPA (batches interleaved for PE/ACT overlap) ----
        with tc.tile_pool(name="pexp", bufs=6) as pexp, \
             tc.tile_pool(name="cnorm", bufs=2) as cnorm, \
             tc.tile_pool(name="ps4", bufs=3, space="PSUM") as ps4, \
             tc.tile_pool(name="ps4b", bufs=1, space="PSUM") as ps4b, \
             tc.tile_pool(name="ps4c", bufs=1, space="PSUM") as ps4c:
            for step in range(B * QT):
                b, j = step % B, step // B
                base = b * S
                ctxp = {0: ps4c.tile([65, 512], F32, name=f"ctxA{b}", tag=f"ctxA{b}"),
                        1: ps4c.tile([65, 512], F32, name=f"ctxB{b}", tag=f"ctxB{b}")}
                nkb = 4 * (j + 1)
                for kb in range(nkb):
                    m = kb - 4 * j
                    c0 = 128 * m if m >= 0 else 0
                    w = 512 - c0
                    qcol0 = base + 512 * j + c0
                    koff = base + kb * 128
                    for hi, r0 in ((0, 0), (1, 64)):
                        sc = ps4.tile([128, 512], F32, name="sc", tag="sc")
                        nc.tensor.matmul(
                            sc[:, 0:w],
                            kT[r0 : r0 + 64, koff : koff + 128],
                            qT[r0 : r0 + 64, qcol0 : qcol0 + w],
                            start=True, stop=True)
                        p = pexp.tile([128, 512], BF16, name=f"p{hi}", tag=f"p{hi}")
                        nc.scalar.activation(
                            p[:, c0:512], sc[:, 0:w],
                            mybir.ActivationFunctionType.Exp, scale=0.125)
                        if m >= 0:
                            nc.vector.tensor_mul(p[:, c0 : c0 + 128], p[:, c0 : c0 + 128], dmaskT)
                        vcol = (b * NKB + kb) * 130 + hi * 65
                        nc.tensor.matmul(
                            ctxp[hi][:, c0:512],
                            vvf[:, vcol : vcol + 65],
                            p[:, c0:512],
                            start=(kb == 0), stop=(kb == nkb - 1),
                            skip_group_check=True)
                # normalize: ctx / denom (denom = row 64 of ctx psum)
                cch = b * QT + j
                csl = slice(cch * CHUNK, (cch + 1) * CHUNK)
                for hi, r0 in ((0, 0), (1, 64)):
                    den_s = cnorm.tile([1, 512], F32, tag="den_s")
                    nc.vector.tensor_copy(den_s, ctxp[hi][64:65, :])
                    rec = cnorm.tile([1, 512], F32, tag="rec")
                    nc.vector.reciprocal_approx_fast(out=rec, in_=den_s)
                    recb = cnorm.tile([1, 512], BF16, tag="recb")
                    nc.vector.tensor_copy(recb, rec)
                    bc = ps4b.tile([64, 512], F32, name="bc", tag="bc")
                    nc.tensor.matmul(bc, ones1[0:1, 0:64], recb, start=True, stop=True)
                    bcs = cnorm.tile([64, 512], BF16, tag="bcs")
                    nc.vector.tensor_copy(bcs, bc)
                    nc.vector.tensor_mul(
                        ctx_sb[r0 : r0 + 64, csl], ctxp[hi][0:64, :], bcs)
                nc.sync.dma_start(
                    out=a2a_in[cch * 128 : (cch + 1) * 128, :], in_=ctx_sb[:, csl])

        if DEBUG:
            nc.sync.dma_start(out=qTd[:, :], in_=qT)
            nc.sync.dma_start(out=kTd[:, :], in_=kT)
            nc.sync.dma_start(out=vvd[:, :], in_=vv.rearrange("p b c -> p (b c)"))
            nc.sync.dma_start(out=ctxd[:, :], in_=ctx_sb)
            nc.sync.dma_start(out=invd[:, :], in_=inv_all)

        # ---- A2A: head-sharded ctx -> token-sharded ctx ----
        nc.gpsimd.collective_compute(
            "AllToAll", mybir.AluOpType.bypass,
            replica_groups=[list(range(NC))],
            ins=[a2a_in.ap().opt()], outs=[a2a_out.ap().opt()])

        # ---- phase 3: out-projection on own token chunk ----
        with tc.tile_pool(name="ctxgp", bufs=1) as ctxgp, \
             tc.tile_pool(name="outp", bufs=3) as outp, \
             tc.tile_pool(name="ps6", bufs=2, space="PSUM") as ps6:
            ctxg = ctxgp.tile([128, 8, CHUNK], BF16, tag="ctxg")
            nc.sync.dma_start(
                out=ctxg, in_=a2a_out.ap().rearrange("(cb p) t -> p cb t", p=128))
            for tl in range(4):
                pso = {nh: ps6.tile([128, 512], F32, name=f"op{nh}", tag=f"op{nh}")
                       for nh in range(2)}
                for nh in range(2):
                    for cb in range(8):
                        nc.tensor.matmul(
                            pso[nh],
                            ctxg[:, cb, tl * 128 : (tl + 1) * 128],
                            wo_sb[:, cb, nh * 512 : (nh + 1) * 512],
                            start=(cb == 0), stop=(cb == 7))
                ost = outp.tile([128, 1024], F32, tag="ost")
                nc.scalar.copy(ost[:, 0:512], pso[0])
                nc.scalar.copy(ost[:, 512:1024], pso[1])
                nc.sync.dma_start(out=out_d[tl * 128 : (tl + 1) * 128, :], in_=ost)

    nc.compile()
    return nc


def _head_cols(h, deinterleave):
    base = h * DH
    if deinterleave:
        return np.concatenate([base + np.arange(0, DH, 2), base + np.arange(1, DH, 2)])
    return base + np.arange(DH)


def _make_tables():
    inv_freq = 1.0 / (THETA ** (np.arange(0, DH, 2) / DH))   # [32]
    ang = np.arange(S)[:, None] * inv_freq[None, :]          # [2048, 32]
    ch = np.cos(ang).T.astype(np.float32)                    # [32, 2048]
    sh = np.sin(ang).T.astype(np.float32)
    cosb = np.tile(np.concatenate([ch, ch, ch, ch], axis=0), (1, B))
    sinb = np.tile(np.concatenate([-sh, sh, -sh, sh], axis=0), (1, B))
    kk, qq = np.meshgrid(np.arange(128), np.arange(128), indexing="ij")
    dmask = (kk <= qq).astype(np.float32)
    bf = ml_dtypes.bfloat16
    return cosb.astype(bf), sinb.astype(bf), dmask.astype(bf)


def _in_maps(inputs):
    bf = ml_dtypes.bfloat16
    x = np.ascontiguousarray(inputs["x"], dtype=np.float32)
    norm_w = np.asarray(inputs["norm_w"], dtype=np.float32)
    wq = np.asarray(inputs["wq"], dtype=np.float32) * norm_w[:, None]
    wk = np.asarray(inputs["wk"], dtype=np.float32) * norm_w[:, None]
    wv = np.asarray(inputs["wv"], dtype=np.float32) * norm_w[:, None]
    wo = np.ascontiguousarray(inputs["wo"], dtype=np.float32).astype(bf)

    xT = np.ascontiguousarray(x.reshape(TOK, D).T.astype(bf))  # [1024, 4096]
    cosb, sinb, dmask = _make_tables()

    maps = []
    for c in range(NC):
        qcols = np.concatenate([_head_cols(2 * c, True), _head_cols(2 * c + 1, True)])
        vcols = np.concatenate([_head_cols(2 * c, False), _head_cols(2 * c + 1, False)])
        maps.append({
            "xc": np.ascontiguousarray(xT[:, c * CHUNK : (c + 1) * CHUNK]),
            "xtb": xT,
            "wqc": np.ascontiguousarray(wq[:, qcols].astype(bf)),
            "wkc": np.ascontiguousarray(wk[:, qcols].astype(bf)),
            "wvc": np.ascontiguousarray(wv[:, vcols].astype(bf)),
            "wo": wo,
            "cosb": cosb,
            "sinb": sinb,
            "dmask": dmask,
        })
    return maps


def _run(inputs, trace=False):
    if "nc" not in _CACHE:
        _CACHE["nc"] = _build()
    nc = _CACHE["nc"]
    res = run_bass_kernel_spmd(nc, _in_maps(inputs), core_ids=list(range(NC)),
                               trace=trace)
    chunks = [res.results[c]["out"] for c in range(NC)]
    out = np.concatenate(chunks, axis=0).reshape(B, S, D).astype(np.float32)
    return out, res


def kernel(**inputs) -> np.ndarray:
    out, _ = _run(inputs, trace=False)
    return out


# revision 10
# speedup vs baseline: 1.3934x; 1.2264x over previous
"""Distributed Trainium2 kernel for nn_Attention (RMSNorm + QKV + RoPE +
causal SDPA + out-proj) over 8 NeuronCores.

v5 strategy (head-sharded QKV, no input collective): every core receives
the FULL x (transposed, bf16) and projects q/k/v for only ITS two heads
over all 4096 tokens -- identical FLOPs to v4's local-chunk/all-heads
split, but the 3MB AllToAll after the projections disappears.

  phase 0: RMSNorm statistics for the core's own 512-token chunk only
           (squares on DVE, column-sum via ones-matmul); 1/rms is
           AllGather'ed (2KB) -- this tiny collective doubles as the
           launch-skew rendezvous and completes under the x DMA stream.
  phase 1: q/k for the 2 heads, all tokens ([128,512] psum tiles,
           8 k-tiles each); RoPE applied at psum-evict time with
           cos/sin tables pre-scaled by 1/rms (RoPE commutes with
           per-token scalars).  v is projected directly TRANSPOSED
           (tokens on partitions: lhsT = x-block, rhs = wv) so SDPA
           needs no PE transposes; 1/rms applied per-partition at the
           v evict.  norm_w is folded into the weights on the host;
           all weights arrive pre-cast to bf16 (no on-chip casts).
  phase 2: causal SDPA in S^T layout for the two heads (v4 scheme:
           scores^T = K_blk.T @ Q, exp on ScalarE without max
           subtraction, diag-block masking, AV with a ones column in
           V so the softmax denominator rides in the same psum tile,
           deferred division).  Batches interleaved for PE/ACT overlap.
  A2A:     context head-sharded -> token-sharded (1MB/rank).
  phase 3: out-projection for the core's own 512-token chunk.
Host does layout-only prep (transpose+bf16 cast, head-column
permutation, norm_w fold, constant RoPE/mask tables) and final concat.
"""
import sys

sys.path.insert(0, "/opt/trn_rl_repo")

import numpy as np
import ml_dtypes
from contextlib import ExitStack

import concourse.bass as bass
import concourse.mybir as mybir
import concourse.tile as tile
from concourse import bacc
from concourse.bass_utils import run_bass_kernel_spmd

F32 = mybir.dt.float32
BF16 = mybir.dt.bfloat16

B, S, D, H, DH = 2, 2048, 1024, 16, 64
NC = 8
TOK = B * S            # 4096
CHUNK = TOK // NC      # 512
EPS = 1.1920929e-07
THETA = 10000.0
NKB = S // 128         # key blocks per batch: 16
QT = S // 512          # q tiles per batch: 4

_CACHE = {}
DEBUG = False


def _build():
    nc = bacc.Bacc("TRN2", target_bir_lowering=False, debug=False, num_devices=NC)

    xc_d = nc.dram_tensor("xc", [D, CHUNK], BF16, kind="ExternalInput")
    xtb_d = nc.dram_tensor("xtb", [D, TOK], BF16, kind="ExternalInput")
    wq_d = nc.dram_tensor("wqc", [D, 128], BF16, kind="ExternalInput")
    wk_d = nc.dram_tensor("wkc", [D, 128], BF16, kind="ExternalInput")
    wv_d = nc.dram_tensor("wvc", [D, 128], BF16, kind="ExternalInput")
    wo_d = nc.dram_tensor("wo", [D, D], BF16, kind="ExternalInput")
    cos_d = nc.dram_tensor("cosb", [128, TOK], BF16, kind="ExternalInput")
    sin_d = nc.dram_tensor("sinb", [128, TOK], BF16, kind="ExternalInput")
    msk_d = nc.dram_tensor("dmask", [128, 128], BF16, kind="ExternalInput")
    out_d = nc.dram_tensor("out", [CHUNK, D], F32, kind="ExternalOutput")

    if DEBUG:
        qTd = nc.dram_tensor("qTd", [128, TOK], BF16, kind="ExternalOutput")
        kTd = nc.dram_tensor("kTd", [128, TOK], BF16, kind="ExternalOutput")
        vvd = nc.dram_tensor("vvd", [128, 32 * 130], BF16, kind="ExternalOutput")
        ctxd = nc.dram_tensor("ctxd", [128, TOK], BF16, kind="ExternalOutput")
        invd = nc.dram_tensor("invd", [1, TOK], F32, kind="ExternalOutput")
    ag_in = nc.dram_tensor("ag_in", [1, CHUNK], F32)
    ag_out = nc.dram_tensor("ag_out", [1, TOK], F32)
    a2a_in = nc.dram_tensor("a2a_in", [NC * 128, CHUNK], BF16)
    a2a_out = nc.dram_tensor("a2a_out", [NC * 128, CHUNK], BF16)

    with tile.TileContext(nc) as tc, ExitStack() as ctx:
        pp = ctx.enter_context(tc.tile_pool(name="persist", bufs=1))

        # ---- persistent tiles ----
        qT = pp.tile([128, TOK], BF16, tag="qT")
        kT = pp.tile([128, TOK], BF16, tag="kT")
        # vv[:, blk, :] = [h0 dims 0:64 | ones | h1 dims 65:129 | ones]
        vv = pp.tile([128, B * NKB, 130], BF16, tag="vv")
        cosS = pp.tile([128, TOK], BF16, tag="cosS")
        sinS = pp.tile([128, TOK], BF16, tag="sinS")
        ctx_sb = pp.tile([128, TOK], BF16, tag="ctx_sb")
        wq_sb = pp.tile([128, 8, 128], BF16, tag="wq_sb")
        wk_sb = pp.tile([128, 8, 128], BF16, tag="wk_sb")
        wv_sb = pp.tile([128, 8, 128], BF16, tag="wv_sb")
        wo_sb = pp.tile([128, 8, 1024], BF16, tag="wo_sb")
        inv_all = pp.tile([1, TOK], F32, tag="inv_all")
        invT = pp.tile([128, B * NKB], F32, tag="invT")
        dmaskT = pp.tile([128, 128], BF16, tag="dmaskT")
        ones128 = pp.tile([128, 1], BF16, tag="ones128")
        ones1 = pp.tile([1, 128], BF16, tag="ones1")

        nc.vector.memset(ones128, 1.0)
        nc.vector.memset(ones1, 1.0)
        nc.gpsimd.memset(vv[:, :, 64:65], 1.0)
        nc.gpsimd.memset(vv[:, :, 129:130], 1.0)

        xs_cm = tc.tile_pool(name="xspool", bufs=1)
        xs_pool = xs_cm.__enter__()
        xs = xs_pool.tile([128, 8, TOK], BF16, tag="xs")
        xcs = xs_pool.tile([128, 8, CHUNK], BF16, tag="xcs")

        # ---- input DMAs (big 3D-AP transfers; issue in consumption order)
        nc.sync.dma_start(
            out=xcs, in_=xc_d.ap().rearrange("(k p) t -> p k t", p=128))
        nc.scalar.dma_start(
            out=wq_sb, in_=wq_d.ap().rearrange("(k p) d -> p k d", p=128))
        nc.scalar.dma_start(
            out=wk_sb, in_=wk_d.ap().rearrange("(k p) d -> p k d", p=128))
        nc.scalar.dma_start(
            out=wv_sb, in_=wv_d.ap().rearrange("(k p) d -> p k d", p=128))
        for tt in range(NC):
            sl = slice(tt * CHUNK, (tt + 1) * CHUNK)
            nc.sync.dma_start(
                out=xs[:, :, sl],
                in_=xtb_d.ap()[:, sl].rearrange("(k p) t -> p k t", p=128))
        nc.scalar.dma_start(out=cosS, in_=cos_d[:, :])
        nc.scalar.dma_start(out=sinS, in_=sin_d[:, :])
        nc.scalar.dma_start(out=dmaskT, in_=msk_d[:, :])
        nc.gpsimd.dma_start(
            out=wo_sb, in_=wo_d.ap().rearrange("(k p) d -> p k d", p=128))

        # ---- phase 0: RMSNorm stats of own chunk + AllGather of 1/rms ----
        with tc.tile_pool(name="rms", bufs=2) as rms_pool, \
             tc.tile_pool(name="psrms", bufs=1, space="PSUM") as psrms:
            ssq = psrms.tile([1, CHUNK], F32, tag="ssq")
            for kt in range(8):
                xsq = rms_pool.tile([128, CHUNK], BF16, tag="xsq")
                nc.vector.tensor_mul(xsq, xcs[:, kt, :], xcs[:, kt, :])
                nc.tensor.matmul(ssq, ones128, xsq, start=(kt == 0), stop=(kt == 7))
            eps_t = rms_pool.tile([1, 1], F32, tag="eps_t")
            nc.vector.memset(eps_t, float(EPS))
            rstd = rms_pool.tile([1, CHUNK], F32, tag="rstd")
            nc.scalar.activation(rstd, ssq, mybir.ActivationFunctionType.Sqrt,
                                 bias=eps_t[0:1, 0:1], scale=1.0 / D)
            inv = rms_pool.tile([1, CHUNK], F32, tag="inv")
            nc.vector.reciprocal_approx_fast(out=inv, in_=rstd)
            nc.scalar.dma_start(out=ag_in[:, :], in_=inv)

        nc.gpsimd.collective_compute(
            "AllGather", mybir.AluOpType.bypass,
            replica_groups=[list(range(NC))],
            ins=[ag_in.ap().opt()], outs=[ag_out.ap().opt()])
        nc.scalar.dma_start(out=inv_all, in_=ag_out[:, :])
        nc.scalar.dma_start(
            out=invT, in_=ag_out.ap().rearrange("a (k p) -> p (a k)", p=128))

        # scale RoPE tables by 1/rms (per-token column scalar commutes with
        # RoPE): rb = broadcast of 1/rms over partitions, then in-place muls
        with tc.tile_pool(name="rbp", bufs=2) as rbpool:
            for tt in range(NC):
                sl = slice(tt * CHUNK, (tt + 1) * CHUNK)
                rb = rbpool.tile([128, CHUNK], F32, tag="rb")
                nc.gpsimd.partition_broadcast(rb[:, :], inv_all[0:1, sl])
                nc.vector.tensor_mul(cosS[:, sl], cosS[:, sl], rb)
                nc.vector.tensor_mul(sinS[:, sl], sinS[:, sl], rb)

            # ---- phase 1: QKV for the 2 heads over all tokens ----
            with tc.tile_pool(name="pstage", bufs=6) as pstage, \
                 tc.tile_pool(name="psqk", bufs=3, space="PSUM") as psqk, \
                 tc.tile_pool(name="psv", bufs=2, space="PSUM") as psv:
                for tt in range(NC):
                    sl = slice(tt * CHUNK, (tt + 1) * CHUNK)
                    for wsb, dst in ((wq_sb, qT), (wk_sb, kT)):
                        acc = psqk.tile([128, CHUNK], F32, tag="accqk")
                        for kt in range(8):
                            nc.tensor.matmul(acc, wsb[:, kt, :], xs[:, kt, sl],
                                             start=(kt == 0), stop=(kt == 7))
                        t = pstage.tile([128, CHUNK], BF16, tag="t")
                        nc.scalar.copy(t, acc)
                        sw = pstage.tile([128, CHUNK], BF16, tag="sw")
                        for a, b2 in ((0, 32), (64, 96)):
                            nc.gpsimd.dma_start(out=sw[a : a + 32, :], in_=t[b2 : b2 + 32, :])
                            nc.gpsimd.dma_start(out=sw[b2 : b2 + 32, :], in_=t[a : a + 32, :])
                        t1 = pstage.tile([128, CHUNK], BF16, tag="t1")
                        nc.vector.tensor_mul(t1, t, cosS[:, sl])
                        nc.vector.tensor_mul(sw, sw, sinS[:, sl])
                        nc.vector.tensor_add(dst[:, sl], t1, sw)
                    # v transposed directly: lhsT = x token-block (stationary)
                    for vb in range(4):
                        blk = tt * 4 + vb
                        c0 = tt * CHUNK + vb * 128
                        accv = psv.tile([128, 128], F32, tag="accv")
                        for kt in range(8):
                            nc.tensor.matmul(accv, xs[:, kt, c0 : c0 + 128],
                                             wv_sb[:, kt, :],
                                             start=(kt == 0), stop=(kt == 7))
                        nc.vector.tensor_scalar_mul(
                            vv[:, blk, 0:64], accv[:, 0:64], invT[:, blk : blk + 1])
                        nc.vector.tensor_scalar_mul(
                            vv[:, blk, 65:129], accv[:, 64:128], invT[:, blk : blk + 1])

        xs_cm.__exit__(None, None, None)
        vvf = vv.rearrange("p blk c -> p (blk c)")

        # ---- phase 2: SDPA (batches interleaved for PE/ACT overlap).
        # Both heads' scores for a key-block live in one 2-bank psum pair
        # tile; full blocks take a single 1024-wide exp.
        with tc.tile_pool(name="pexp", bufs=4) as pexp, \
             tc.tile_pool(name="cnorm", bufs=2) as cnorm, \
             tc.tile_pool(name="ps4", bufs=2, space="PSUM") as ps4, \
             tc.tile_pool(name="ps4c", bufs=1, space="PSUM") as ps4c:
            for step in range(B * QT):
                b, j = step % B, step // B
                base = b * S
                ctxp = {0: ps4c.tile([65, 512], F32, name=f"ctxA{b}", tag=f"ctxA{b}"),
                        1: ps4c.tile([65, 512], F32, name=f"ctxB{b}", tag=f"ctxB{b}")}
                nkb = 4 * (j + 1)
                for kb in range(nkb):
                    m = kb - 4 * j
                    c0 = 128 * m if m >= 0 else 0
                    w = 512 - c0
                    qcol0 = base + 512 * j + c0
                    koff = base + kb * 128
                    sc = ps4.tile([128, 1024], F32, name="sc", tag="sc")
                    p = pexp.tile([128, 1024], BF16, name="p", tag="p")
                    for hi, r0 in ((0, 0), (1, 64)):
                        nc.tensor.matmul(
                            sc[:, hi * 512 + c0 : hi * 512 + 512],
                            kT[r0 : r0 + 64, koff : koff + 128],
                            qT[r0 : r0 + 64, qcol0 : qcol0 + w],
                            start=True, stop=True)
                    if m >= 0:
                        for hi in (0, 1):
                            nc.scalar.activation(
                                p[:, hi * 512 + c0 : hi * 512 + 512],
                                sc[:, hi * 512 + c0 : hi * 512 + 512],
                                mybir.ActivationFunctionType.Exp, scale=0.125)
                            nc.vector.tensor_mul(
                                p[:, hi * 512 + c0 : hi * 512 + c0 + 128],
                                p[:, hi * 512 + c0 : hi * 512 + c0 + 128], dmaskT)
                    else:
                        nc.scalar.activation(
                            p, sc, mybir.ActivationFunctionType.Exp, scale=0.125)
                    for hi in (0, 1):
                        vcol = (b * NKB + kb) * 130 + hi * 65
                        nc.tensor.matmul(
                            ctxp[hi][:, c0:512],
                            vvf[:, vcol : vcol + 65],
                            p[:, hi * 512 + c0 : hi * 512 + 512],
                            start=(kb == 0), stop=(kb == nkb - 1),
                            skip_group_check=True)
                # normalize: ctx / denom (denom = row 64 of ctx psum)
                cch = b * QT + j
                csl = slice(cch * CHUNK, (cch + 1) * CHUNK)
                for hi, r0 in ((0, 0), (1, 64)):
                    den_s = cnorm.tile([1, 512], F32, tag="den_s")
                    nc.vector.tensor_copy(den_s, ctxp[hi][64:65, :])
                    rec = cnorm.tile([1, 512], F32, tag="rec")
                    nc.vector.reciprocal_approx_fast(out=rec, in_=den_s)
                    recb = cnorm.tile([1, 512], BF16, tag="recb")
                    nc.vector.tensor_copy(recb, rec)
                    bcs = cnorm.tile([64, 512], BF16, tag="bcs")
                    nc.gpsimd.partition_broadcast(bcs[:, :], recb[0:1, :])
                    nc.vector.tensor_mul(
                        ctx_sb[r0 : r0 + 64, csl], ctxp[hi][0:64, :], bcs)
                nc.sync.dma_start(
                    out=a2a_in[cch * 128 : (cch + 1) * 128, :], in_=ctx_sb[:, csl])

        if DEBUG:
            nc.sync.dma_start(out=qTd[:, :], in_=qT)
            nc.sync.dma_start(out=kTd[:, :], in_=kT)
            nc.sync.dma_start(out=vvd[:, :], in_=vv.rearrange("p b c -> p (b c)"))
            nc.sync.dma_start(out=ctxd[:, :], in_=ctx_sb)
            nc.sync.dma_start(out=invd[:, :], in_=inv_all)

        # ---- A2A: head-sharded ctx -> token-sharded ctx ----
        nc.gpsimd.collective_compute(
            "AllToAll", mybir.AluOpType.bypass,
            replica_groups=[list(range(NC))],
            ins=[a2a_in.ap().opt()], outs=[a2a_out.ap().opt()])

        # ---- phase 3: out-projection on own token chunk ----
        with tc.tile_pool(name="ctxgp", bufs=1) as ctxgp, \
             tc.tile_pool(name="outp", bufs=3) as outp, \
             tc.tile_pool(name="ps6", bufs=2, space="PSUM") as ps6:
            ctxg = ctxgp.tile([128, 8, CHUNK], BF16, tag="ctxg")
            nc.sync.dma_start(
                out=ctxg, in_=a2a_out.ap().rearrange("(cb p) t -> p cb t", p=128))
            for tl in range(4):
                pso = {nh: ps6.tile([128, 512], F32, name=f"op{nh}", tag=f"op{nh}")
                       for nh in range(2)}
                for nh in range(2):
                    for cb in range(8):
                        nc.tensor.matmul(
                            pso[nh],
                            ctxg[:, cb, tl * 128 : (tl + 1) * 128],
                            wo_sb[:, cb, nh * 512 : (nh + 1) * 512],
                            start=(cb == 0), stop=(cb == 7))
                ost = outp.tile([128, 1024], F32, tag="ost")
                nc.scalar.copy(ost[:, 0:512], pso[0])
                nc.scalar.copy(ost[:, 512:1024], pso[1])
                nc.sync.dma_start(out=out_d[tl * 128 : (tl + 1) * 128, :], in_=ost)

    nc.compile()
    return nc


def _head_cols(h, deinterleave):
    base = h * DH
    if deinterleave:
        return np.concatenate([base + np.arange(0, DH, 2), base + np.arange(1, DH, 2)])
    return base + np.arange(DH)


def _make_tables():
    inv_freq = 1.0 / (THETA ** (np.arange(0, DH, 2) / DH))   # [32]
    ang = np.arange(S)[:, None] * inv_freq[None, :]          # [2048, 32]
    ch = np.cos(ang).T.astype(np.float32)                    # [32, 2048]
    sh = np.sin(ang).T.astype(np.float32)
    cosb = np.tile(np.concatenate([ch, ch, ch, ch], axis=0), (1, B))
    sinb = np.tile(np.concatenate([-sh, sh, -sh, sh], axis=0), (1, B))
    kk, qq = np.meshgrid(np.arange(128), np.arange(128), indexing="ij")
    dmask = (kk <= qq).astype(np.float32)
    bf = ml_dtypes.bfloat16
    return cosb.astype(bf), sinb.astype(bf), dmask.astype(bf)


def _in_maps(inputs):
    bf = ml_dtypes.bfloat16
    x = np.ascontiguousarray(inputs["x"], dtype=np.float32)
    norm_w = np.asarray(inputs["norm_w"], dtype=np.float32)
    wq = np.asarray(inputs["wq"], dtype=np.float32) * norm_w[:, None]
    wk = np.asarray(inputs["wk"], dtype=np.float32) * norm_w[:, None]
    wv = np.asarray(inputs["wv"], dtype=np.float32) * norm_w[:, None]
    wo = np.ascontiguousarray(inputs["wo"], dtype=np.float32).astype(bf)

    xT = np.ascontiguousarray(x.reshape(TOK, D).T.astype(bf))  # [1024, 4096]
    cosb, sinb, dmask = _make_tables()

    maps = []
    for c in range(NC):
        qcols = np.concatenate([_head_cols(2 * c, True), _head_cols(2 * c + 1, True)])
        vcols = np.concatenate([_head_cols(2 * c, False), _head_cols(2 * c + 1, False)])
        maps.append({
            "xc": np.ascontiguousarray(xT[:, c * CHUNK : (c + 1) * CHUNK]),
            "xtb": xT,
            "wqc": np.ascontiguousarray(wq[:, qcols].astype(bf)),
            "wkc": np.ascontiguousarray(wk[:, qcols].astype(bf)),
            "wvc": np.ascontiguousarray(wv[:, vcols].astype(bf)),
            "wo": wo,
            "cosb": cosb,
            "sinb": sinb,
            "dmask": dmask,
        })
    return maps


def _run(inputs, trace=False):
    if "nc" not in _CACHE:
        _CACHE["nc"] = _build()
    nc = _CACHE["nc"]
    res = run_bass_kernel_spmd(nc, _in_maps(inputs), core_ids=list(range(NC)),
                               trace=trace)
    chunks = [res.results[c]["out"] for c in range(NC)]
    out = np.concatenate(chunks, axis=0).reshape(B, S, D).astype(np.float32)
    return out, res


def kernel(**inputs) -> np.ndarray:
    out, _ = _run(inputs, trace=False)
    return out
